# revision 49
# baseline (speedup 1.0000x reference)
"""TRN2 Bass kernel for nn_MultiHeadAttention (B=4, S=2048, D=512, H=8).

Computation (per reference):
  v_in = LN(seq_v) ; q = seq_q@W1.T ; k = seq_k@W2.T ; v = v_in@W3.T
  scores[b,h,i,j] = k_i . q_j ; attn = softmax_j(scores) ; out = attn @ v
  out = LN(out + v_in)

Sharding (zero-communication): core c -> (batch b=c//2, i-half=c%2).
Each core computes all 8 heads for its 1024 output rows (the "i" index,
which indexes K rows), needing full q/v (all j) for its batch and the
i-half slice of k. The j axis is permuted host-side (own half first) so
one SPMD program serves all cores; softmax over j is permutation
invariant and the residual rows are j-tiles 0..7 by construction.

Key techniques (final: fused pipeline, 2-byte matmuls everywhere):
  - ALL side work (projections, v-proj, LN stats, finalize) is
    interleaved into the attention jt-loops so the Tensor engine never
    idles (measured: f32r matmuls run ~1.75 cycles/row on HW while
    fp16/bf16 run ~1/row with back-to-back overlap).
  - q/k path in fp16 (10 mantissa bits: score-logit rounding ~5e-3,
    well inside tolerance; bf16 scores FAIL at 3.3e-2); v path
    (svT/W3/vaug/p/outT/vinres/sv) in bf16 - those errors average out
    across the softmax sum. Scores accumulate in f32 PSUM.
  - Scalar engine runs ONLY the softmax exp (the hard floor: 16.8M
    exps x ~1.1ns incl 287ns/instr PSUM-access overhead ~ 145us);
    LN rsqrt is a DVE Newton iteration, copies are DVE casts. One
    activation-table load total.
  - GpSimd tensor_scalar is a ~15ns/element DSP soft-loop - never
    used; GpSimd cannot touch PSUM at all.
  - LN of seq_v folded into the v-projection: v = rstd*(sv@W3g.T) -
    (mu*rstd)*g3 (+c3), one DVE scalar_tensor_tensor per tile.
  - scores computed transposed [j, i], two heads row-packed (K=64 at
    PE rows 0/64); softmax denominator via ones-column in v (M=65 PV).
  - PV lags scores by 2 jt-tiles so the PE never waits on the exp.
  - startup: first k-proj quarter gated on only ~0.5MB (weights and
    first chunks DMA'd in halves); the bulk DMA queue is release-gated
    behind the critical prefix so it cannot starve it.
  - finalize split into per-(head-pair, i-tile-pair) panel units that
    depend only on single attention blocks -> transposes/divides
    pipeline into later blocks; only the last 2 panels + 4 row-LNs
    trail the final block.
"""

import numpy as np

B, S, D, H = 4, 2048, 512, 8
HD = D // H  # 64
EPS = 1e-5
NCORES = 8
IH = S // 2          # 1024 output rows per core
NT = S // 128        # 16 j token-tiles
ITILES = IH // 128   # 8 i-tiles
DT = D // 128        # 4 d-tiles (head pairs)
ET = D // 128        # 4 e-tiles (contraction)
NC = 4               # input chunks (512 tokens each)

_cache = {}


def _build(has_gamma: bool, has_beta: bool):
    import concourse.bacc as bacc
    import concourse.mybir as mybir
    import concourse.tile as tile
    from concourse.masks import make_identity

    f32 = mybir.dt.float32
    f32r = mybir.dt.float32r
    bf16 = mybir.dt.bfloat16
    f16 = mybir.dt.float16
    i32 = mybir.dt.int32
    Alu = mybir.AluOpType
    Act = mybir.ActivationFunctionType

    nc = bacc.Bacc(None, target_bir_lowering=False)

    sqT = nc.dram_tensor("sqT", [128, ET, S], f16, kind="ExternalInput")
    skT = nc.dram_tensor("skT", [128, ET, IH], f16, kind="ExternalInput")
    svT = nc.dram_tensor("svT", [128, ET, S], bf16, kind="ExternalInput")
    sv = nc.dram_tensor("sv", [128, NT, 512], bf16, kind="ExternalInput")
    w1T = nc.dram_tensor("w1T", [128, ET, D], f16, kind="ExternalInput")
    w2T = nc.dram_tensor("w2T", [128, ET, D], f16, kind="ExternalInput")
    w3gT = nc.dram_tensor("w3gT", [128, ET, D], bf16, kind="ExternalInput")
    g3 = nc.dram_tensor("g3", [1, D], f32, kind="ExternalInput")
    c3v = nc.dram_tensor("c3v", [1, D], f32, kind="ExternalInput")
    gamma = nc.dram_tensor("gamma", [1, D], f32, kind="ExternalInput")
    beta = nc.dram_tensor("beta", [1, D], f32, kind="ExternalInput")
    out = nc.dram_tensor("out", [128, ITILES, D], f32, kind="ExternalOutput")

    def bcast(dram_ap):
        import concourse.bass as bass

        return bass.AP(
            tensor=dram_ap.tensor,
            offset=dram_ap.offset,
            ap=[[0, 128], [1, D]],
        )

    ts = lambda i, sz: slice(i * sz, (i + 1) * sz)

    with tile.TileContext(nc) as tc:
        with (
            tc.tile_pool(name="const", bufs=1) as const,
            tc.tile_pool(name="persist", bufs=1) as persist,
            tc.tile_pool(name="wts", bufs=1) as wts,
            tc.tile_pool(name="qstage", bufs=4) as qstage,
            tc.tile_pool(name="kstage", bufs=2) as kstage,
            tc.tile_pool(name="svs", bufs=2) as svs,
            tc.tile_pool(name="svts", bufs=4) as svts,
            tc.tile_pool(name="ppool", bufs=4) as ppool,
            tc.tile_pool(name="fin", bufs=4) as fin,
            tc.tile_pool(name="vtmp", bufs=2) as vtmp,
            tc.tile_pool(name="sps", bufs=2, space="PSUM") as sps,
            tc.tile_pool(name="ops", bufs=1, space="PSUM") as ops,
            tc.tile_pool(name="wps", bufs=2, space="PSUM") as wps,
        ):
            # ---- DMA issue (sync queue, in needed-order) ----
            # first k/q projection quarter is gated on only ~1MB: weights
            # and the first token chunks are DMA'd in halves
            w2_sb = wts.tile([128, ET, D], f16, tag="w2")
            nc.sync.dma_start(w2_sb[:, :, 0:256], w2T[:, :, 0:256])
            skc = [None] * 2
            skc[0] = kstage.tile([128, ET, 512], f16, tag="skc", name="skc0")
            nc.sync.dma_start(skc[0][:, :, 0:256], skT[:, :, 0:256])
            w1_sb = wts.tile([128, ET, D], f16, tag="w1")
            nc.sync.dma_start(w1_sb[:, :, 0:256], w1T[:, :, 0:256])
            sqc = [None] * NC
            sqc[0] = qstage.tile([128, ET, 512], f16, tag="sqc", name="sqc0")
            nc.sync.dma_start(sqc[0][:, :, 0:256], sqT[:, :, 0:256])
            nc.sync.dma_start(skc[0][:, :, 256:512], skT[:, :, 256:512])
            nc.sync.dma_start(sqc[0][:, :, 256:512], sqT[:, :, 256:512])
            w3_sb = wts.tile([128, ET, D], bf16, tag="w3")
            nc.sync.dma_start(w3_sb, w3gT[:])

            def load_sq(c, eng):
                sqc[c] = qstage.tile(
                    [128, ET, 512], f16, tag="sqc", name="sqcn"
                )
                eng.dma_start(sqc[c], sqT[:, :, ts(c, 512)])

            def load_sk(c, eng):
                skc[c] = kstage.tile(
                    [128, ET, 512], f16, tag="skc", name="skcn"
                )
                eng.dma_start(skc[c], skT[:, :, ts(c, 512)])

            load_sq(1, nc.sync)
            nc.sync.dma_start(w2_sb[:, :, 256:512], w2T[:, :, 256:512])
            nc.sync.dma_start(w1_sb[:, :, 256:512], w1T[:, :, 256:512])

            # sv / svT chunks (gpsimd queue, parallel with sync)
            g3b = const.tile([128, D], f32, tag="g3b")
            nc.gpsimd.dma_start(g3b, bcast(g3[:]))
            if has_gamma:
                gammab = const.tile([128, D], f32, tag="gammab")
                nc.gpsimd.dma_start(gammab, bcast(gamma[:]))
            if has_beta:
                betab = const.tile([128, D], f32, tag="betab")
                nc.gpsimd.dma_start(betab, bcast(beta[:]))
                c3b = const.tile([128, D], f32, tag="c3b")
                nc.gpsimd.dma_start(c3b, bcast(c3v[:]))
            svc_t = [None] * NC
            svtc_t = [None] * NC

            def load_sv(c):
                svc_t[c] = svs.tile([128, 4, 512], bf16, tag="sv", name="svc")
                nc.gpsimd.dma_start(svc_t[c], sv[:, ts(c, 4), :])
                svtc_t[c] = svts.tile(
                    [128, ET, 512], bf16, tag="svt", name="svtc"
                )
                nc.gpsimd.dma_start(svtc_t[c], svT[:, :, ts(c, 512)])

            # delay the bulk queue until the PE-gating sync prefix lands:
            # this copy depends on the sqc0 first-half DMA, so every
            # descriptor below queues behind it
            qgate = const.tile([128, 1], f32, tag="qgate")
            nc.gpsimd.tensor_copy(qgate, sqc[0][:, 0, 0:1])
            load_sv(0)
            load_sv(1)
            load_sq(2, nc.gpsimd)
            load_sv(2)
            load_sq(3, nc.gpsimd)
            load_sv(3)
            load_sk(1, nc.gpsimd)

            # ---- constants ----
            identb = const.tile([128, 128], bf16, tag="identb")
            make_identity(nc, identb)
            # load the Exp table now (dep-free) so exp(0) skips the
            # 1.28us table load on its critical path
            warm = const.tile([128, 1], f32, tag="warm")
            nc.vector.memset(warm, 0.0)
            nc.scalar.activation(warm, warm, Act.Exp)

            # ---- persistent intermediates ----
            qT_sb = persist.tile([128, DT, S], f16, tag="qT")
            kT_sb = persist.tile([128, DT, IH], f16, tag="kT")
            vaug = persist.tile([128, NT, H, 65], bf16, tag="vaug")
            outT_e = persist.tile([65, DT, IH], bf16, tag="outTe")
            outT_o = persist.tile([65, DT, IH], bf16, tag="outTo")
            vinres = persist.tile([128, ITILES, 512], bf16, tag="vinres")
            mu_sb = persist.tile([128, NT], f32, tag="mu")
            var_sb = persist.tile([128, NT], f32, tag="var")
            rstd_sb = persist.tile([128, NT], f32, tag="rstd")
            onesc = const.tile([128, NT * H], f32, tag="onesc")
            nc.vector.memset(onesc, 1.0)
            nc.vector.tensor_copy(
                vaug[:, :, :, 64],
                onesc.rearrange("p (a b) -> p a b", a=NT),
            )

            def newton_rsqrt(dst, src, n):
                # dst = 1/sqrt(src), DVE-only (fast-inverse-sqrt + 2 Newton)
                nc.vector.tensor_scalar(
                    out=dst.bitcast(i32),
                    in0=src.bitcast(i32),
                    scalar1=1,
                    scalar2=None,
                    op0=Alu.logical_shift_right,
                )
                nc.vector.tensor_scalar(
                    out=dst.bitcast(i32),
                    in0=dst.bitcast(i32),
                    scalar1=-1,
                    scalar2=0x5F3759DF,
                    op0=Alu.mult,
                    op1=Alu.add,
                )
                tmp1 = fin.tile([128, n], f32, tag="ntmp", name="ntmp")
                for _ in range(2):
                    nc.vector.tensor_mul(tmp1, dst, dst)
                    nc.vector.tensor_mul(tmp1, tmp1, src)
                    nc.vector.tensor_scalar(
                        out=tmp1,
                        in0=tmp1,
                        scalar1=-0.5,
                        scalar2=1.5,
                        op0=Alu.mult,
                        op1=Alu.add,
                    )
                    nc.vector.tensor_mul(dst, dst, tmp1)

            # ---- work units ----
            def stats_jt(jt):
                # LN statistics for token tile jt (DVE only)
                x = svc_t[jt // 4][:, jt % 4, :]
                st = fin.tile([128, 6], f32, tag="st0")
                nc.vector.bn_stats(st, x)
                mv = fin.tile([128, 2], f32, tag="mv0")
                nc.vector.bn_aggr(mv, st)
                nc.vector.tensor_copy(mu_sb[:, jt : jt + 1], mv[:, 0:1])
                nc.vector.tensor_copy(var_sb[:, jt : jt + 1], mv[:, 1:2])

            def stats_chunk(c):
                # rstd for 4 tiles via Newton; vinres for residual tiles
                ve = fin.tile([128, 4], f32, tag="ve4")
                nc.vector.tensor_scalar_add(ve, var_sb[:, ts(c, 4)], EPS)
                newton_rsqrt(rstd_sb[:, ts(c, 4)], ve, 4)
                for jt in range(4 * c, min(4 * c + 4, ITILES)):
                    x = svc_t[jt // 4][:, jt % 4, :]
                    nc.vector.tensor_scalar(
                        out=vinres[:, jt, :],
                        in0=x,
                        scalar1=mu_sb[:, jt : jt + 1],
                        scalar2=rstd_sb[:, jt : jt + 1],
                        op0=Alu.subtract,
                        op1=Alu.mult,
                    )
                    if has_gamma:
                        nc.vector.tensor_mul(
                            vinres[:, jt, :], vinres[:, jt, :], gammab
                        )
                    if has_beta:
                        nc.vector.tensor_add(
                            vinres[:, jt, :], vinres[:, jt, :], betab
                        )

            def qproj_half(t, jc, h):
                # half-width q projection (startup: smaller DMA gate)
                ps = wps.tile([128, 512], f32, tag="work", name="psqh")
                for e in range(ET):
                    nc.tensor.matmul(
                        ps[:, 0:256],
                        w1_sb[:, e, ts(t, 128)],
                        sqc[jc][:, e, ts(h, 256)],
                        start=(e == 0),
                        stop=(e == ET - 1),
                    )
                nc.vector.tensor_copy(
                    qT_sb[:, t, jc * 512 + h * 256 : jc * 512 + h * 256 + 256],
                    ps[:, 0:256],
                )

            def kproj_half(t, ic, h):
                ps = wps.tile([128, 512], f32, tag="work", name="pskh")
                for e in range(ET):
                    nc.tensor.matmul(
                        ps[:, 0:256],
                        w2_sb[:, e, ts(t, 128)],
                        skc[ic][:, e, ts(h, 256)],
                        start=(e == 0),
                        stop=(e == ET - 1),
                    )
                nc.vector.tensor_copy(
                    kT_sb[:, t, ic * 512 + h * 256 : ic * 512 + h * 256 + 256],
                    ps[:, 0:256],
                )

            def qproj_unit(t, jc):
                # q projection for head-pair t, token chunk jc -> qT (f32r)
                ps = wps.tile([128, 512], f32, tag="work", name="psq")
                for e in range(ET):
                    nc.tensor.matmul(
                        ps,
                        w1_sb[:, e, ts(t, 128)],
                        sqc[jc][:, e, :],
                        start=(e == 0),
                        stop=(e == ET - 1),
                    )
                nc.vector.tensor_copy(qT_sb[:, t, ts(jc, 512)], ps)

            def kproj_unit(t, ic):
                ps = wps.tile([128, 512], f32, tag="work", name="psk")
                for e in range(ET):
                    nc.tensor.matmul(
                        ps,
                        w2_sb[:, e, ts(t, 128)],
                        skc[ic][:, e, :],
                        start=(e == 0),
                        stop=(e == ET - 1),
                    )
                nc.vector.tensor_copy(kT_sb[:, t, ts(ic, 512)], ps)

            def vproj_step(jt):
                # v-projection for token tile jt with LN folded in (DVE)
                svtc = svtc_t[jt // 4]
                ps = wps.tile([128, 512], f32, tag="work", name="psv")
                for e in range(ET):
                    nc.tensor.matmul(
                        ps,
                        svtc[:, e, ts(jt % 4, 128)],
                        w3_sb[:, e, :],
                        start=(e == 0),
                        stop=(e == ET - 1),
                    )
                mr = fin.tile([128, 1], f32, tag="mr")
                nc.vector.tensor_mul(
                    mr, mu_sb[:, jt : jt + 1], rstd_sb[:, jt : jt + 1]
                )
                tA = vtmp.tile([128, 512], f32, tag="tA")
                nc.vector.tensor_scalar_mul(tA, g3b, mr)
                vdst = vaug[:, jt, :, 0:64]
                nc.vector.scalar_tensor_tensor(
                    out=vdst,
                    in0=ps.rearrange("p (h d) -> p h d", h=H),
                    scalar=rstd_sb[:, jt : jt + 1],
                    in1=tA.rearrange("p (h d) -> p h d", h=H),
                    op0=Alu.mult,
                    op1=Alu.subtract,
                )
                if has_beta:
                    nc.vector.tensor_add(
                        vdst, vdst, c3b.rearrange("p (h d) -> p h d", h=H)
                    )

            def fin_panels(t, ita, itb, ys, use_scalar=False):
                # transpose+divide the (t, e/o) panels of i-tiles ita/itb:
                # depends only on block (t, ib(ita)) - pipelines early
                tpf = wps.tile([128, 512], f32, tag="work", name="tpf")
                tp = tpf.bitcast(bf16)
                for c in range(4):
                    it = ita if c < 2 else itb
                    src = outT_e if c % 2 == 0 else outT_o
                    nc.tensor.transpose(
                        tp[:, c * 66 : c * 66 + 65],
                        src[0:65, t, ts(it, 128)],
                        identb[0:65, 0:65],
                    )
                rc = fin.tile([128, 4], f32, tag="rc")
                tp4 = tp[:, 0:264].rearrange("p (c x) -> p c x", c=4)
                nc.vector.reciprocal(rc, tp4[:, :, 64])
                for c in range(4):
                    it = ita if c < 2 else itb
                    off = 0 if c % 2 == 0 else 64
                    col = t * 128 + off
                    if use_scalar:
                        nc.scalar.mul(
                            ys[it][:, col : col + 64],
                            tp[:, c * 66 : c * 66 + 64],
                            rc[:, c : c + 1],
                        )
                    else:
                        nc.vector.tensor_scalar_mul(
                            ys[it][:, col : col + 64],
                            tp[:, c * 66 : c * 66 + 64],
                            rc[:, c : c + 1],
                        )

            def fin_divide(it, grp, tp, y, use_scalar=False):
                # rc = 1/denominator for the 4 panels, then scale into y
                rc = fin.tile([128, 4], f32, tag="rc")
                tp4 = tp[:, 0:264].rearrange("p (c x) -> p c x", c=4)
                nc.vector.reciprocal(rc, tp4[:, :, 64])
                for c in range(4):
                    t = grp * 2 + c // 2
                    off = 0 if c % 2 == 0 else 64
                    col = t * 128 + off
                    if use_scalar:
                        nc.scalar.mul(
                            y[:, col : col + 64],
                            tp[:, c * 66 : c * 66 + 64],
                            rc[:, c : c + 1],
                        )
                    else:
                        nc.vector.tensor_scalar_mul(
                            y[:, col : col + 64],
                            tp[:, c * 66 : c * 66 + 64],
                            rc[:, c : c + 1],
                        )

            def fin_ln(it, y):
                # residual + final LN (DVE/Pool; Newton keeps Scalar free)
                aeng = nc.gpsimd if it % 2 == 0 else nc.vector
                aeng.tensor_add(y, y, vinres[:, it, :])
                st = fin.tile([128, 6], f32, tag="st")
                nc.vector.bn_stats(st, y)
                mv = fin.tile([128, 2], f32, tag="mv")
                nc.vector.bn_aggr(mv, st)
                ve = fin.tile([128, 1], f32, tag="ve")
                nc.vector.tensor_scalar_add(ve, mv[:, 1:2], EPS)
                rstd2 = fin.tile([128, 1], f32, tag="rstd2")
                newton_rsqrt(rstd2, ve, 1)
                nc.vector.tensor_scalar(
                    out=y,
                    in0=y,
                    scalar1=mv[:, 0:1],
                    scalar2=rstd2,
                    op0=Alu.subtract,
                    op1=Alu.mult,
                )
                if has_gamma:
                    nc.vector.tensor_mul(y, y, gammab)
                if has_beta:
                    nc.gpsimd.tensor_add(y, y, betab)
                nc.sync.dma_start(out[:, it, :], y)

            def make_finalize_units(it, use_scalar=False):
                # finalize as 3 schedulable units sharing a y tile
                y = fin.tile([128, 512], f32, tag="y", name="y")

                def u0():
                    tp = fin_transpose(it, 0)
                    fin_divide(it, 0, tp, y, use_scalar)

                def u1():
                    tp = fin_transpose(it, 1)
                    fin_divide(it, 1, tp, y, use_scalar)

                def u2():
                    fin_ln(it, y)

                return [u0, u1, u2]

            # ---- attention block (i-range [i_off, i_off+i_len)) ----
            def attn_block(t, i_off, i_len, extra=None, lag=2):
                isl = slice(i_off, i_off + i_len)
                o_e = ops.tile([65, 512], f32, tag="oe", name="oe")[:, 0:i_len]
                o_o = ops.tile([65, 512], f32, tag="oo", name="oo")[:, 0:i_len]

                def pv(jt, p):
                    nc.tensor.matmul(
                        o_e,
                        vaug[:, jt, 2 * t, :],
                        p[:, 0:i_len],
                        start=(jt == 0),
                        stop=(jt == NT - 1),
                    )
                    nc.tensor.matmul(
                        o_o,
                        vaug[:, jt, 2 * t + 1, :],
                        p[:, i_len : 2 * i_len],
                        start=(jt == 0),
                        stop=(jt == NT - 1),
                    )

                pend = []
                for jt in range(NT):
                    s = sps.tile([128, 1024], f32, tag="s")
                    nc.tensor.matmul(
                        s[:, 0:i_len],
                        qT_sb[0:64, t, ts(jt, 128)],
                        kT_sb[0:64, t, isl],
                        start=True,
                        stop=True,
                    )
                    nc.tensor.matmul(
                        s[:, i_len : 2 * i_len],
                        qT_sb[64:128, t, ts(jt, 128)],
                        kT_sb[64:128, t, isl],
                        start=True,
                        stop=True,
                    )
                    p = ppool.tile([128, 1024], bf16, tag="p")
                    nc.scalar.activation(
                        p[:, 0 : 2 * i_len], s[:, 0 : 2 * i_len], Act.Exp
                    )
                    if extra is not None:
                        for fn in extra.get(jt, ()):
                            fn()
                    pend.append((jt, p))
                    if len(pend) > lag:
                        pv(*pend.pop(0))
                for e in pend:
                    pv(*e)
                nc.vector.tensor_copy(outT_e[:, t, isl], o_e)
                nc.vector.tensor_copy(outT_o[:, t, isl], o_o)

            # ---- schedule ----
            # preamble: minimum PE work to start block (0,0), chunk-0 stats
            kproj_half(0, 0, 0)
            qproj_half(0, 0, 0)
            kproj_half(0, 0, 1)
            qproj_half(0, 0, 1)
            for jt in range(4):
                stats_jt(jt)
            stats_chunk(0)

            # block (0,0): vproj delayed 2 slots behind its pv consumer
            # (lag 3) so a late svT chunk cannot stall the PE FIFO ahead
            # of the score matmuls; stats shift with it
            ex = {jt: [] for jt in range(NT)}
            for jt in range(4, NT):
                ex[min(jt - 2, 13)].append(lambda jt=jt: stats_jt(jt))
            ex[5].append(lambda: stats_chunk(1))
            ex[9].append(lambda: stats_chunk(2))
            ex[13].append(lambda: stats_chunk(3))
            for jt in range(NT):
                ex[min(jt + 2, 15)].append(lambda jt=jt: vproj_step(jt))
            ex[1].insert(0, lambda: qproj_unit(0, 1))
            ex[5].insert(0, lambda: qproj_unit(0, 2))
            ex[9].insert(0, lambda: qproj_unit(0, 3))
            ex[11].insert(0, lambda: kproj_unit(1, 0))
            ex[13].insert(0, lambda: qproj_unit(1, 0))
            ex[14].insert(0, lambda: qproj_unit(1, 1))
            ex[15].insert(0, lambda: qproj_unit(1, 2))
            attn_block(0, 0, 512, ex, lag=3)

            # finalize pipelining: the (t, it-pair) panel units depend only
            # on block (t, ib) of that i-range - spread them across all
            # later blocks; LNs go as soon as all 4 t-panels of an it exist
            ys = {}

            def get_y(it):
                if it not in ys:
                    ys[it] = fin.tile([128, 512], f32, tag="y", name="y")
                return ys[it]

            def P(t, ita, itb, sc=False):
                for it in (ita, itb):
                    get_y(it)
                return lambda: fin_panels(t, ita, itb, ys, sc)

            def LN(it):
                return lambda: fin_ln(it, ys[it])

            # blocks (1..3, 0): carry next block's projections + k ic=1
            for t in range(1, DT):
                ex = {}
                if t + 1 < DT:
                    ex[2] = [lambda t=t: qproj_unit(t, 3)]
                    ex[4] = [lambda t=t: kproj_unit(t + 1, 0)]
                    ex[7] = [lambda t=t: qproj_unit(t + 1, 0)]
                    ex[10] = [lambda t=t: qproj_unit(t + 1, 1)]
                    ex[13] = [lambda t=t: qproj_unit(t + 1, 2)]
                else:
                    ex[2] = [lambda: qproj_unit(3, 3)]
                if t == 1:
                    ex.setdefault(5, []).append(lambda: kproj_unit(0, 1))
                attn_block(t, 0, 512, ex)

            # blocks (0..3, [512:1024]): moved k-proj + finalize pieces
            # fill the Scalar-bound ib1 blocks' PE slack
            exs = [
                {1: [lambda: kproj_unit(1, 1)], 3: [P(0, 0, 1)],
                 5: [P(1, 0, 1)], 7: [P(2, 0, 1)], 9: [P(3, 0, 1)],
                 11: [P(0, 2, 3)], 13: [LN(0)]},
                {1: [lambda: kproj_unit(2, 1)], 3: [P(1, 2, 3)],
                 5: [P(2, 2, 3)], 7: [P(3, 2, 3)], 9: [LN(1)],
                 11: [LN(2)], 13: [LN(3)]},
                {1: [lambda: kproj_unit(3, 1)], 4: [P(0, 4, 5)],
                 8: [P(1, 4, 5)], 12: [P(0, 6, 7)]},
                {4: [P(2, 4, 5)], 8: [P(1, 6, 7)], 12: [P(2, 6, 7)]},
            ]
            for t in range(DT):
                attn_block(t, 512, 512, exs[t])

            # tail: last t-panels (Scalar + DVE in parallel) + final LNs
            P(3, 4, 5, True)()
            P(3, 6, 7, False)()
            for it in range(4, ITILES):
                LN(it)()

    nc.compile()
    return nc


def _bf16():
    import ml_dtypes

    return ml_dtypes.bfloat16


def _to_tiles_T(x, dt=np.float32):
    # [N, 512] -> [128, 4, N] : out[p, t, n] = x[n, 128*t + p]
    n = x.shape[0]
    return np.ascontiguousarray(
        x.T.reshape(ET, 128, n).transpose(1, 0, 2).astype(dt)
    )


def _w_tiles(w, dt=np.float32):
    # [512, 512] (e, d) -> [128, 4, 512] : out[p, t, d] = w[128*t + p, d]
    return np.ascontiguousarray(
        w.reshape(ET, 128, D).transpose(1, 0, 2).astype(dt)
    )


def kernel(seq_k, seq_q, seq_v, W1, W2, W3, gamma, beta, _trace=False):
    seq_k = np.asarray(seq_k, dtype=np.float32)
    seq_q = np.asarray(seq_q, dtype=np.float32)
    seq_v = np.asarray(seq_v, dtype=np.float32)
    W1 = np.asarray(W1, dtype=np.float32)
    W2 = np.asarray(W2, dtype=np.float32)
    W3 = np.asarray(W3, dtype=np.float32)
    gamma = np.asarray(gamma, dtype=np.float32)
    beta = np.asarray(beta, dtype=np.float32)

    has_gamma = bool(np.any(gamma != 1.0))
    has_beta = bool(np.any(beta != 0.0))

    key = (has_gamma, has_beta)
    if key not in _cache:
        _cache[key] = _build(has_gamma, has_beta)
    nc = _cache[key]

    from concourse import bass_utils

    bf = _bf16()
    W3g = W3 * gamma[None, :]  # W3g[d, e] = W3[d, e] * gamma[e]
    g3v = np.ascontiguousarray((W3 @ gamma)[None, :], dtype=np.float32)
    c3vv = np.ascontiguousarray((W3 @ beta)[None, :], dtype=np.float32)
    w1t = _w_tiles(np.ascontiguousarray(W1.T), np.float16)
    w2t = _w_tiles(np.ascontiguousarray(W2.T), np.float16)
    w3t = _w_tiles(np.ascontiguousarray(W3g.T), bf)
    gam = np.ascontiguousarray(gamma[None, :], dtype=np.float32)
    bet = np.ascontiguousarray(beta[None, :], dtype=np.float32)

    in_maps = []
    for c in range(NCORES):
        b, half = divmod(c, 2)
        lo, hi = half * IH, half * IH + IH
        perm = np.r_[lo:hi, 0:lo, hi:S]
        sq = seq_q[b][perm]
        svp = seq_v[b][perm]
        sk = seq_k[b, lo:hi]
        in_maps.append(
            {
                "sqT": _to_tiles_T(sq, np.float16),
                "skT": _to_tiles_T(sk, np.float16),
                "svT": _to_tiles_T(svp, bf),
                "sv": np.ascontiguousarray(
                    svp.reshape(NT, 128, 512).transpose(1, 0, 2).astype(bf)
                ),
                "w1T": w1t,
                "w2T": w2t,
                "w3gT": w3t,
                "g3": g3v,
                "c3v": c3vv,
                "gamma": gam,
                "beta": bet,
            }
        )

    res = bass_utils.run_bass_kernel_spmd(
        nc, in_maps, core_ids=list(range(NCORES)), trace=_trace
    )
    global _last_run
    _last_run = res

    full = np.empty((B, S, D), dtype=np.float32)
    for c in range(NCORES):
        b, half = divmod(c, 2)
        o = res.results[c]["out"]  # [128, 8, 512]
        full[b, half * IH : (half + 1) * IH] = o.transpose(1, 0, 2).reshape(
            IH, D
        )
    return full


_last_run = None


# revision 50
# speedup vs baseline: 1.0025x; 1.0025x over previous
"""TRN2 Bass kernel for nn_MultiHeadAttention (B=4, S=2048, D=512, H=8).

Computation (per reference):
  v_in = LN(seq_v) ; q = seq_q@W1.T ; k = seq_k@W2.T ; v = v_in@W3.T
  scores[b,h,i,j] = k_i . q_j ; attn = softmax_j(scores) ; out = attn @ v
  out = LN(out + v_in)

Sharding (zero-communication): core c -> (batch b=c//2, i-half=c%2).
Each core computes all 8 heads for its 1024 output rows (the "i" index,
which indexes K rows), needing full q/v (all j) for its batch and the
i-half slice of k. The j axis is permuted host-side (own half first) so
one SPMD program serves all cores; softmax over j is permutation
invariant and the residual rows are j-tiles 0..7 by construction.

Key techniques (final: fused pipeline, 2-byte matmuls everywhere):
  - ALL side work (projections, v-proj, LN stats, finalize) is
    interleaved into the attention jt-loops so the Tensor engine never
    idles (measured: f32r matmuls run ~1.75 cycles/row on HW while
    fp16/bf16 run ~1/row with back-to-back overlap).
  - q/k path in fp16 (10 mantissa bits: score-logit rounding ~5e-3,
    well inside tolerance; bf16 scores FAIL at 3.3e-2); v path
    (svT/W3/vaug/p/outT/vinres/sv) in bf16 - those errors average out
    across the softmax sum. Scores accumulate in f32 PSUM.
  - Scalar engine runs ONLY the softmax exp (the hard floor: 16.8M
    exps x ~1.1ns incl 287ns/instr PSUM-access overhead ~ 145us);
    LN rsqrt is a DVE Newton iteration, copies are DVE casts. One
    activation-table load total.
  - GpSimd tensor_scalar is a ~15ns/element DSP soft-loop - never
    used; GpSimd cannot touch PSUM at all.
  - LN of seq_v folded into the v-projection: v = rstd*(sv@W3g.T) -
    (mu*rstd)*g3 (+c3), one DVE scalar_tensor_tensor per tile.
  - scores computed transposed [j, i], two heads row-packed (K=64 at
    PE rows 0/64); softmax denominator via ones-column in v (M=65 PV).
  - PV lags scores by 2 jt-tiles so the PE never waits on the exp.
  - startup: first k-proj quarter gated on only ~0.5MB (weights and
    first chunks DMA'd in halves); the bulk DMA queue is release-gated
    behind the critical prefix so it cannot starve it.
  - finalize split into per-(head-pair, i-tile-pair) panel units that
    depend only on single attention blocks -> transposes/divides
    pipeline into later blocks; only the last 2 panels + 4 row-LNs
    trail the final block.
"""

import numpy as np

B, S, D, H = 4, 2048, 512, 8
HD = D // H  # 64
EPS = 1e-5
NCORES = 8
IH = S // 2          # 1024 output rows per core
NT = S // 128        # 16 j token-tiles
ITILES = IH // 128   # 8 i-tiles
DT = D // 128        # 4 d-tiles (head pairs)
ET = D // 128        # 4 e-tiles (contraction)
NC = 4               # input chunks (512 tokens each)

_cache = {}


def _build(has_gamma: bool, has_beta: bool):
    import concourse.bacc as bacc
    import concourse.mybir as mybir
    import concourse.tile as tile
    from concourse.masks import make_identity

    f32 = mybir.dt.float32
    f32r = mybir.dt.float32r
    bf16 = mybir.dt.bfloat16
    f16 = mybir.dt.float16
    i32 = mybir.dt.int32
    Alu = mybir.AluOpType
    Act = mybir.ActivationFunctionType

    nc = bacc.Bacc(None, target_bir_lowering=False)

    sqT = nc.dram_tensor("sqT", [128, ET, S], f16, kind="ExternalInput")
    skT = nc.dram_tensor("skT", [128, ET, IH], f16, kind="ExternalInput")
    svT = nc.dram_tensor("svT", [128, ET, S], bf16, kind="ExternalInput")
    sv = nc.dram_tensor("sv", [128, NT, 512], bf16, kind="ExternalInput")
    w1T = nc.dram_tensor("w1T", [128, ET, D], f16, kind="ExternalInput")
    w2T = nc.dram_tensor("w2T", [128, ET, D], f16, kind="ExternalInput")
    w3gT = nc.dram_tensor("w3gT", [128, ET, D], bf16, kind="ExternalInput")
    g3 = nc.dram_tensor("g3", [1, D], f32, kind="ExternalInput")
    c3v = nc.dram_tensor("c3v", [1, D], f32, kind="ExternalInput")
    gamma = nc.dram_tensor("gamma", [1, D], f32, kind="ExternalInput")
    beta = nc.dram_tensor("beta", [1, D], f32, kind="ExternalInput")
    out = nc.dram_tensor("out", [128, ITILES, D], f32, kind="ExternalOutput")

    def bcast(dram_ap):
        import concourse.bass as bass

        return bass.AP(
            tensor=dram_ap.tensor,
            offset=dram_ap.offset,
            ap=[[0, 128], [1, D]],
        )

    ts = lambda i, sz: slice(i * sz, (i + 1) * sz)

    with tile.TileContext(nc) as tc:
        with (
            tc.tile_pool(name="const", bufs=1) as const,
            tc.tile_pool(name="persist", bufs=1) as persist,
            tc.tile_pool(name="wts", bufs=1) as wts,
            tc.tile_pool(name="qstage", bufs=4) as qstage,
            tc.tile_pool(name="kstage", bufs=2) as kstage,
            tc.tile_pool(name="svs", bufs=2) as svs,
            tc.tile_pool(name="svts", bufs=4) as svts,
            tc.tile_pool(name="ppool", bufs=4) as ppool,
            tc.tile_pool(name="fin", bufs=4) as fin,
            tc.tile_pool(name="vtmp", bufs=2) as vtmp,
            tc.tile_pool(name="sps", bufs=2, space="PSUM") as sps,
            tc.tile_pool(name="ops", bufs=1, space="PSUM") as ops,
            tc.tile_pool(name="wps", bufs=2, space="PSUM") as wps,
        ):
            # ---- DMA issue (sync queue, in needed-order) ----
            # first k/q projection quarter is gated on only ~1MB: weights
            # and the first token chunks are DMA'd in halves
            w2_sb = wts.tile([128, ET, D], f16, tag="w2")
            nc.sync.dma_start(w2_sb[:, :, 0:256], w2T[:, :, 0:256])
            skc = [None] * 2
            skc[0] = kstage.tile([128, ET, 512], f16, tag="skc", name="skc0")
            nc.sync.dma_start(skc[0][:, :, 0:256], skT[:, :, 0:256])
            w1_sb = wts.tile([128, ET, D], f16, tag="w1")
            nc.sync.dma_start(w1_sb[:, :, 0:256], w1T[:, :, 0:256])
            sqc = [None] * NC
            sqc[0] = qstage.tile([128, ET, 512], f16, tag="sqc", name="sqc0")
            nc.sync.dma_start(sqc[0][:, :, 0:256], sqT[:, :, 0:256])
            nc.sync.dma_start(skc[0][:, :, 256:512], skT[:, :, 256:512])
            nc.sync.dma_start(sqc[0][:, :, 256:512], sqT[:, :, 256:512])
            w3_sb = wts.tile([128, ET, D], bf16, tag="w3")
            nc.sync.dma_start(w3_sb, w3gT[:])

            def load_sq(c, eng):
                sqc[c] = qstage.tile(
                    [128, ET, 512], f16, tag="sqc", name="sqcn"
                )
                eng.dma_start(sqc[c], sqT[:, :, ts(c, 512)])

            def load_sk(c, eng):
                skc[c] = kstage.tile(
                    [128, ET, 512], f16, tag="skc", name="skcn"
                )
                eng.dma_start(skc[c], skT[:, :, ts(c, 512)])

            load_sq(1, nc.sync)
            nc.sync.dma_start(w2_sb[:, :, 256:512], w2T[:, :, 256:512])
            nc.sync.dma_start(w1_sb[:, :, 256:512], w1T[:, :, 256:512])

            # sv / svT chunks (gpsimd queue, parallel with sync)
            g3b = const.tile([128, D], f32, tag="g3b")
            nc.gpsimd.dma_start(g3b, bcast(g3[:]))
            if has_gamma:
                gammab = const.tile([128, D], f32, tag="gammab")
                nc.gpsimd.dma_start(gammab, bcast(gamma[:]))
            if has_beta:
                betab = const.tile([128, D], f32, tag="betab")
                nc.gpsimd.dma_start(betab, bcast(beta[:]))
                c3b = const.tile([128, D], f32, tag="c3b")
                nc.gpsimd.dma_start(c3b, bcast(c3v[:]))
            svc_t = [None] * NC
            svtc_t = [None] * NC

            def load_sv(c):
                svc_t[c] = svs.tile([128, 4, 512], bf16, tag="sv", name="svc")
                nc.gpsimd.dma_start(svc_t[c], sv[:, ts(c, 4), :])
                svtc_t[c] = svts.tile(
                    [128, ET, 512], bf16, tag="svt", name="svtc"
                )
                nc.gpsimd.dma_start(svtc_t[c], svT[:, :, ts(c, 512)])

            # delay the bulk queue until the PE-gating sync prefix lands:
            # this copy depends on the sqc0 first-half DMA, so every
            # descriptor below queues behind it
            qgate = const.tile([128, 1], f32, tag="qgate")
            nc.gpsimd.tensor_copy(qgate, sqc[0][:, 0, 0:1])
            load_sv(0)
            load_sv(1)
            load_sq(2, nc.gpsimd)
            load_sv(2)
            load_sq(3, nc.gpsimd)
            load_sv(3)
            load_sk(1, nc.gpsimd)

            # ---- constants ----
            identb = const.tile([128, 128], bf16, tag="identb")
            make_identity(nc, identb)
            # load the Exp table now (dep-free) so exp(0) skips the
            # 1.28us table load on its critical path
            warm = const.tile([128, 1], f32, tag="warm")
            nc.vector.memset(warm, 0.0)
            nc.scalar.activation(warm, warm, Act.Exp)

            # ---- persistent intermediates ----
            qT_sb = persist.tile([128, DT, S], f16, tag="qT")
            kT_sb = persist.tile([128, DT, IH], f16, tag="kT")
            vaug = persist.tile([128, NT, H, 65], bf16, tag="vaug")
            outT_e = persist.tile([65, DT, IH], bf16, tag="outTe")
            outT_o = persist.tile([65, DT, IH], bf16, tag="outTo")
            vinres = persist.tile([128, ITILES, 512], bf16, tag="vinres")
            mu_sb = persist.tile([128, NT], f32, tag="mu")
            var_sb = persist.tile([128, NT], f32, tag="var")
            rstd_sb = persist.tile([128, NT], f32, tag="rstd")
            onesc = const.tile([128, NT * H], f32, tag="onesc")
            nc.vector.memset(onesc, 1.0)
            nc.vector.tensor_copy(
                vaug[:, :, :, 64],
                onesc.rearrange("p (a b) -> p a b", a=NT),
            )

            def newton_rsqrt(dst, src, n):
                # dst = 1/sqrt(src), DVE-only (fast-inverse-sqrt + 2 Newton)
                nc.vector.tensor_scalar(
                    out=dst.bitcast(i32),
                    in0=src.bitcast(i32),
                    scalar1=1,
                    scalar2=None,
                    op0=Alu.logical_shift_right,
                )
                nc.vector.tensor_scalar(
                    out=dst.bitcast(i32),
                    in0=dst.bitcast(i32),
                    scalar1=-1,
                    scalar2=0x5F3759DF,
                    op0=Alu.mult,
                    op1=Alu.add,
                )
                tmp1 = fin.tile([128, n], f32, tag="ntmp", name="ntmp")
                for _ in range(2):
                    nc.vector.tensor_mul(tmp1, dst, dst)
                    nc.vector.tensor_mul(tmp1, tmp1, src)
                    nc.vector.tensor_scalar(
                        out=tmp1,
                        in0=tmp1,
                        scalar1=-0.5,
                        scalar2=1.5,
                        op0=Alu.mult,
                        op1=Alu.add,
                    )
                    nc.vector.tensor_mul(dst, dst, tmp1)

            # ---- work units ----
            def stats_jt(jt):
                # LN statistics for token tile jt (DVE only)
                x = svc_t[jt // 4][:, jt % 4, :]
                st = fin.tile([128, 6], f32, tag="st0")
                nc.vector.bn_stats(st, x)
                mv = fin.tile([128, 2], f32, tag="mv0")
                nc.vector.bn_aggr(mv, st)
                nc.vector.tensor_copy(mu_sb[:, jt : jt + 1], mv[:, 0:1])
                nc.vector.tensor_copy(var_sb[:, jt : jt + 1], mv[:, 1:2])

            def stats_chunk(c):
                # rstd for 4 tiles via Newton; vinres for residual tiles
                ve = fin.tile([128, 4], f32, tag="ve4")
                nc.vector.tensor_scalar_add(ve, var_sb[:, ts(c, 4)], EPS)
                newton_rsqrt(rstd_sb[:, ts(c, 4)], ve, 4)
                for jt in range(4 * c, min(4 * c + 4, ITILES)):
                    x = svc_t[jt // 4][:, jt % 4, :]
                    nc.vector.tensor_scalar(
                        out=vinres[:, jt, :],
                        in0=x,
                        scalar1=mu_sb[:, jt : jt + 1],
                        scalar2=rstd_sb[:, jt : jt + 1],
                        op0=Alu.subtract,
                        op1=Alu.mult,
                    )
                    if has_gamma:
                        nc.vector.tensor_mul(
                            vinres[:, jt, :], vinres[:, jt, :], gammab
                        )
                    if has_beta:
                        nc.vector.tensor_add(
                            vinres[:, jt, :], vinres[:, jt, :], betab
                        )

            def qproj_half(t, jc, h):
                # half-width q projection (startup: smaller DMA gate)
                ps = wps.tile([128, 512], f32, tag="work", name="psqh")
                for e in range(ET):
                    nc.tensor.matmul(
                        ps[:, 0:256],
                        w1_sb[:, e, ts(t, 128)],
                        sqc[jc][:, e, ts(h, 256)],
                        start=(e == 0),
                        stop=(e == ET - 1),
                    )
                nc.vector.tensor_copy(
                    qT_sb[:, t, jc * 512 + h * 256 : jc * 512 + h * 256 + 256],
                    ps[:, 0:256],
                )

            def kproj_half(t, ic, h):
                ps = wps.tile([128, 512], f32, tag="work", name="pskh")
                for e in range(ET):
                    nc.tensor.matmul(
                        ps[:, 0:256],
                        w2_sb[:, e, ts(t, 128)],
                        skc[ic][:, e, ts(h, 256)],
                        start=(e == 0),
                        stop=(e == ET - 1),
                    )
                nc.vector.tensor_copy(
                    kT_sb[:, t, ic * 512 + h * 256 : ic * 512 + h * 256 + 256],
                    ps[:, 0:256],
                )

            def qproj_unit(t, jc):
                # q projection for head-pair t, token chunk jc -> qT (f32r)
                ps = wps.tile([128, 512], f32, tag="work", name="psq")
                for e in range(ET):
                    nc.tensor.matmul(
                        ps,
                        w1_sb[:, e, ts(t, 128)],
                        sqc[jc][:, e, :],
                        start=(e == 0),
                        stop=(e == ET - 1),
                    )
                nc.vector.tensor_copy(qT_sb[:, t, ts(jc, 512)], ps)

            def kproj_unit(t, ic):
                ps = wps.tile([128, 512], f32, tag="work", name="psk")
                for e in range(ET):
                    nc.tensor.matmul(
                        ps,
                        w2_sb[:, e, ts(t, 128)],
                        skc[ic][:, e, :],
                        start=(e == 0),
                        stop=(e == ET - 1),
                    )
                nc.vector.tensor_copy(kT_sb[:, t, ts(ic, 512)], ps)

            def vproj_step(jt):
                # v-projection for token tile jt with LN folded in (DVE)
                svtc = svtc_t[jt // 4]
                ps = wps.tile([128, 512], f32, tag="work", name="psv")
                for e in range(ET):
                    nc.tensor.matmul(
                        ps,
                        svtc[:, e, ts(jt % 4, 128)],
                        w3_sb[:, e, :],
                        start=(e == 0),
                        stop=(e == ET - 1),
                    )
                mr = fin.tile([128, 1], f32, tag="mr")
                nc.vector.tensor_mul(
                    mr, mu_sb[:, jt : jt + 1], rstd_sb[:, jt : jt + 1]
                )
                tA = vtmp.tile([128, 512], f32, tag="tA")
                nc.vector.tensor_scalar_mul(tA, g3b, mr)
                vdst = vaug[:, jt, :, 0:64]
                nc.vector.scalar_tensor_tensor(
                    out=vdst,
                    in0=ps.rearrange("p (h d) -> p h d", h=H),
                    scalar=rstd_sb[:, jt : jt + 1],
                    in1=tA.rearrange("p (h d) -> p h d", h=H),
                    op0=Alu.mult,
                    op1=Alu.subtract,
                )
                if has_beta:
                    nc.vector.tensor_add(
                        vdst, vdst, c3b.rearrange("p (h d) -> p h d", h=H)
                    )

            def fin_panels(t, ita, itb, ys, use_scalar=False):
                # transpose+divide the (t, e/o) panels of i-tiles ita/itb:
                # depends only on block (t, ib(ita)) - pipelines early
                tpf = wps.tile([128, 512], f32, tag="work", name="tpf")
                tp = tpf.bitcast(bf16)
                for c in range(4):
                    it = ita if c < 2 else itb
                    src = outT_e if c % 2 == 0 else outT_o
                    nc.tensor.transpose(
                        tp[:, c * 66 : c * 66 + 65],
                        src[0:65, t, ts(it, 128)],
                        identb[0:65, 0:65],
                    )
                rc = fin.tile([128, 4], f32, tag="rc")
                tp4 = tp[:, 0:264].rearrange("p (c x) -> p c x", c=4)
                nc.vector.reciprocal(rc, tp4[:, :, 64])
                for c in range(4):
                    it = ita if c < 2 else itb
                    off = 0 if c % 2 == 0 else 64
                    col = t * 128 + off
                    if use_scalar:
                        nc.scalar.mul(
                            ys[it][:, col : col + 64],
                            tp[:, c * 66 : c * 66 + 64],
                            rc[:, c : c + 1],
                        )
                    else:
                        nc.vector.tensor_scalar_mul(
                            ys[it][:, col : col + 64],
                            tp[:, c * 66 : c * 66 + 64],
                            rc[:, c : c + 1],
                        )

            def fin_divide(it, grp, tp, y, use_scalar=False):
                # rc = 1/denominator for the 4 panels, then scale into y
                rc = fin.tile([128, 4], f32, tag="rc")
                tp4 = tp[:, 0:264].rearrange("p (c x) -> p c x", c=4)
                nc.vector.reciprocal(rc, tp4[:, :, 64])
                for c in range(4):
                    t = grp * 2 + c // 2
                    off = 0 if c % 2 == 0 else 64
                    col = t * 128 + off
                    if use_scalar:
                        nc.scalar.mul(
                            y[:, col : col + 64],
                            tp[:, c * 66 : c * 66 + 64],
                            rc[:, c : c + 1],
                        )
                    else:
                        nc.vector.tensor_scalar_mul(
                            y[:, col : col + 64],
                            tp[:, c * 66 : c * 66 + 64],
                            rc[:, c : c + 1],
                        )

            def fin_ln(it, y):
                # residual + final LN (DVE/Pool; Newton keeps Scalar free)
                aeng = nc.gpsimd if it % 2 == 0 else nc.vector
                aeng.tensor_add(y, y, vinres[:, it, :])
                st = fin.tile([128, 6], f32, tag="st")
                nc.vector.bn_stats(st, y)
                mv = fin.tile([128, 2], f32, tag="mv")
                nc.vector.bn_aggr(mv, st)
                ve = fin.tile([128, 1], f32, tag="ve")
                nc.vector.tensor_scalar_add(ve, mv[:, 1:2], EPS)
                rstd2 = fin.tile([128, 1], f32, tag="rstd2")
                newton_rsqrt(rstd2, ve, 1)
                nc.vector.tensor_scalar(
                    out=y,
                    in0=y,
                    scalar1=mv[:, 0:1],
                    scalar2=rstd2,
                    op0=Alu.subtract,
                    op1=Alu.mult,
                )
                if has_gamma:
                    nc.vector.tensor_mul(y, y, gammab)
                if has_beta:
                    nc.gpsimd.tensor_add(y, y, betab)
                nc.sync.dma_start(out[:, it, :], y)

            def make_finalize_units(it, use_scalar=False):
                # finalize as 3 schedulable units sharing a y tile
                y = fin.tile([128, 512], f32, tag="y", name="y")

                def u0():
                    tp = fin_transpose(it, 0)
                    fin_divide(it, 0, tp, y, use_scalar)

                def u1():
                    tp = fin_transpose(it, 1)
                    fin_divide(it, 1, tp, y, use_scalar)

                def u2():
                    fin_ln(it, y)

                return [u0, u1, u2]

            # ---- attention block (i-range [i_off, i_off+i_len)) ----
            def attn_block(t, i_off, i_len, extra=None, lag=2):
                isl = slice(i_off, i_off + i_len)
                o_e = ops.tile([65, 512], f32, tag="oe", name="oe")[:, 0:i_len]
                o_o = ops.tile([65, 512], f32, tag="oo", name="oo")[:, 0:i_len]

                def pv(jt, p):
                    nc.tensor.matmul(
                        o_e,
                        vaug[:, jt, 2 * t, :],
                        p[:, 0:i_len],
                        start=(jt == 0),
                        stop=(jt == NT - 1),
                    )
                    nc.tensor.matmul(
                        o_o,
                        vaug[:, jt, 2 * t + 1, :],
                        p[:, i_len : 2 * i_len],
                        start=(jt == 0),
                        stop=(jt == NT - 1),
                    )

                pend = []
                for jt in range(NT):
                    s = sps.tile([128, 1024], f32, tag="s")
                    nc.tensor.matmul(
                        s[:, 0:i_len],
                        qT_sb[0:64, t, ts(jt, 128)],
                        kT_sb[0:64, t, isl],
                        start=True,
                        stop=True,
                    )
                    nc.tensor.matmul(
                        s[:, i_len : 2 * i_len],
                        qT_sb[64:128, t, ts(jt, 128)],
                        kT_sb[64:128, t, isl],
                        start=True,
                        stop=True,
                    )
                    p = ppool.tile([128, 1024], bf16, tag="p")
                    nc.scalar.activation(
                        p[:, 0 : 2 * i_len], s[:, 0 : 2 * i_len], Act.Exp
                    )
                    if extra is not None:
                        for fn in extra.get(jt, ()):
                            fn()
                    pend.append((jt, p))
                    if len(pend) > lag:
                        pv(*pend.pop(0))
                for e in pend:
                    pv(*e)
                nc.vector.tensor_copy(outT_e[:, t, isl], o_e)
                nc.vector.tensor_copy(outT_o[:, t, isl], o_o)

            # ---- schedule ----
            # preamble: minimum PE work to start block (0,0), chunk-0 stats
            kproj_half(0, 0, 0)
            qproj_half(0, 0, 0)
            kproj_half(0, 0, 1)
            qproj_half(0, 0, 1)
            for jt in range(4):
                stats_jt(jt)
            stats_chunk(0)

            # block (0,0): vproj delayed 2 slots behind its pv consumer
            # (lag 3) so a late svT chunk cannot stall the PE FIFO ahead
            # of the score matmuls; stats shift with it
            ex = {jt: [] for jt in range(NT)}
            for jt in range(4, NT):
                ex[min(jt - 2, 13)].append(lambda jt=jt: stats_jt(jt))
            ex[5].append(lambda: stats_chunk(1))
            ex[9].append(lambda: stats_chunk(2))
            ex[13].append(lambda: stats_chunk(3))
            for jt in range(NT):
                ex[min(jt + 2, 15)].append(lambda jt=jt: vproj_step(jt))
            ex[1].insert(0, lambda: qproj_unit(0, 1))
            ex[5].insert(0, lambda: qproj_unit(0, 2))
            ex[9].insert(0, lambda: qproj_unit(0, 3))
            ex[11].insert(0, lambda: kproj_unit(1, 0))
            ex[13].insert(0, lambda: qproj_unit(1, 0))
            ex[14].insert(0, lambda: qproj_unit(1, 1))
            ex[15].insert(0, lambda: qproj_unit(1, 2))
            attn_block(0, 0, 512, ex, lag=3)

            # finalize pipelining: the (t, it-pair) panel units depend only
            # on block (t, ib) of that i-range - spread them across all
            # later blocks; LNs go as soon as all 4 t-panels of an it exist
            ys = {}

            def get_y(it):
                if it not in ys:
                    ys[it] = fin.tile([128, 512], f32, tag="y", name="y")
                return ys[it]

            def P(t, ita, itb, sc=False):
                for it in (ita, itb):
                    get_y(it)
                return lambda: fin_panels(t, ita, itb, ys, sc)

            def LN(it):
                return lambda: fin_ln(it, ys[it])

            # blocks (1..3, 0): carry next block's projections + k ic=1
            for t in range(1, DT):
                ex = {}
                if t + 1 < DT:
                    ex[2] = [lambda t=t: qproj_unit(t, 3)]
                    ex[4] = [lambda t=t: kproj_unit(t + 1, 0)]
                    ex[7] = [lambda t=t: qproj_unit(t + 1, 0)]
                    ex[10] = [lambda t=t: qproj_unit(t + 1, 1)]
                    ex[13] = [lambda t=t: qproj_unit(t + 1, 2)]
                else:
                    ex[2] = [lambda: qproj_unit(3, 3)]
                    ex[4] = [lambda: kproj_unit(2, 1)]
                    ex[7] = [lambda: kproj_unit(3, 1)]
                ex.setdefault(5, []).append(lambda t=t: kproj_unit(t - 1, 1))
                ex.setdefault(8, []).append(P(t - 1, 0, 1))
                ex.setdefault(12, []).append(P(t - 1, 2, 3))
                attn_block(t, 0, 512, ex)

            # blocks (0..3, [512:1024]) with interleaved finalize pieces
            exs = [
                {2: [P(3, 0, 1)], 7: [P(3, 2, 3)], 12: [LN(0)]},
                {2: [LN(1)], 5: [LN(2)], 8: [LN(3)], 11: [P(0, 4, 5)],
                 14: [P(0, 6, 7)]},
                {4: [P(1, 4, 5)], 10: [P(1, 6, 7)]},
                {4: [P(2, 4, 5)], 10: [P(2, 6, 7)]},
            ]
            for t in range(DT):
                attn_block(t, 512, 512, exs[t])

            # tail: last t-panels (Scalar + DVE in parallel) + final LNs
            P(3, 4, 5, True)()
            P(3, 6, 7, False)()
            for it in range(4, ITILES):
                LN(it)()

    nc.compile()
    return nc


def _bf16():
    import ml_dtypes

    return ml_dtypes.bfloat16


def _to_tiles_T(x, dt=np.float32):
    # [N, 512] -> [128, 4, N] : out[p, t, n] = x[n, 128*t + p]
    n = x.shape[0]
    return np.ascontiguousarray(
        x.T.reshape(ET, 128, n).transpose(1, 0, 2).astype(dt)
    )


def _w_tiles(w, dt=np.float32):
    # [512, 512] (e, d) -> [128, 4, 512] : out[p, t, d] = w[128*t + p, d]
    return np.ascontiguousarray(
        w.reshape(ET, 128, D).transpose(1, 0, 2).astype(dt)
    )


def kernel(seq_k, seq_q, seq_v, W1, W2, W3, gamma, beta, _trace=False):
    seq_k = np.asarray(seq_k, dtype=np.float32)
    seq_q = np.asarray(seq_q, dtype=np.float32)
    seq_v = np.asarray(seq_v, dtype=np.float32)
    W1 = np.asarray(W1, dtype=np.float32)
    W2 = np.asarray(W2, dtype=np.float32)
    W3 = np.asarray(W3, dtype=np.float32)
    gamma = np.asarray(gamma, dtype=np.float32)
    beta = np.asarray(beta, dtype=np.float32)

    has_gamma = bool(np.any(gamma != 1.0))
    has_beta = bool(np.any(beta != 0.0))

    key = (has_gamma, has_beta)
    if key not in _cache:
        _cache[key] = _build(has_gamma, has_beta)
    nc = _cache[key]

    from concourse import bass_utils

    bf = _bf16()
    W3g = W3 * gamma[None, :]  # W3g[d, e] = W3[d, e] * gamma[e]
    g3v = np.ascontiguousarray((W3 @ gamma)[None, :], dtype=np.float32)
    c3vv = np.ascontiguousarray((W3 @ beta)[None, :], dtype=np.float32)
    w1t = _w_tiles(np.ascontiguousarray(W1.T), np.float16)
    w2t = _w_tiles(np.ascontiguousarray(W2.T), np.float16)
    w3t = _w_tiles(np.ascontiguousarray(W3g.T), bf)
    gam = np.ascontiguousarray(gamma[None, :], dtype=np.float32)
    bet = np.ascontiguousarray(beta[None, :], dtype=np.float32)

    in_maps = []
    for c in range(NCORES):
        b, half = divmod(c, 2)
        lo, hi = half * IH, half * IH + IH
        perm = np.r_[lo:hi, 0:lo, hi:S]
        sq = seq_q[b][perm]
        svp = seq_v[b][perm]
        sk = seq_k[b, lo:hi]
        in_maps.append(
            {
                "sqT": _to_tiles_T(sq, np.float16),
                "skT": _to_tiles_T(sk, np.float16),
                "svT": _to_tiles_T(svp, bf),
                "sv": np.ascontiguousarray(
                    svp.reshape(NT, 128, 512).transpose(1, 0, 2).astype(bf)
                ),
                "w1T": w1t,
                "w2T": w2t,
                "w3gT": w3t,
                "g3": g3v,
                "c3v": c3vv,
                "gamma": gam,
                "beta": bet,
            }
        )

    res = bass_utils.run_bass_kernel_spmd(
        nc, in_maps, core_ids=list(range(NCORES)), trace=_trace
    )
    global _last_run
    _last_run = res

    full = np.empty((B, S, D), dtype=np.float32)
    for c in range(NCORES):
        b, half = divmod(c, 2)
        o = res.results[c]["out"]  # [128, 8, 512]
        full[b, half * IH : (half + 1) * IH] = o.transpose(1, 0, 2).reshape(
            IH, D
        )
    return full


_last_run = None
